# revision 27
# baseline (speedup 1.0000x reference)
"""Trainium2 Bass kernel for nn_LookWhereDownstream (LookWhere two-stage ViT).

Sharding: data-parallel over batch; cores 0-3 compute images 0-3, cores 4-7
run duplicates (same SPMD program). All compute on device; host does only
weight folding/casting and pure input relayout (hi-res im2col permutation).

Layout: residual stream feature-major xT=[768, NT] f32 in SBUF. Matmuls
consume xT directly (contraction on partitions); LN stats via ones-matmuls on
bf16 casts; per-token scalars broadcast across partitions with K=1 PE matmuls
(f32r). Attention computes scoresT=[keys, queries] per head (exp without max
subtraction - scores are bounded ~3) so softmax normalization folds into the
o copyback. Top-k via exact rank counting (compare+accumulate), gather as a
one-hot matmul; gathered-token order doesn't affect the cls output
(attention is permutation-invariant over keys).
"""
import numpy as np
import ml_dtypes

import concourse.bacc as bacc
import concourse.mybir as mybir
from concourse.tile import TileContext
from concourse.bass_utils import run_bass_kernel_spmd

F32, BF16, F32R = mybir.dt.float32, mybir.dt.bfloat16, mybir.dt.float32r
AF = mybir.ActivationFunctionType
OP = mybir.AluOpType
bfloat16 = ml_dtypes.bfloat16

D, NH, HD = 768, 12, 64
K_SEL = 512
HR, LR, P = 518, 154, 14
G_HR, G_LR, R = 37, 11, 4
N_HR, N_LR = G_HR * G_HR, G_LR * G_LR     # 1369, 121
NT_SEL = 5 + N_LR                          # 126
NT_EXT = 5 + K_SEL                         # 517
NCLS = 1000
KD = D // 128                              # 6
DH = 4 * D
EPS = 1e-6
N_CORES = 8
BATCH = 4


def _nsplit(n, cap=512):
    out, s = [], 0
    while s < n:
        c = min(cap, n - s)
        out.append((s, c))
        s += c
    return out


def _msplit(n):
    return _nsplit(n, 128)


def _resize_matrix(n_in, n_out):
    scale = n_out / n_in
    sample_f = (np.arange(n_out) + 0.5) / scale - 0.5
    kscale = max(1.0 / scale, 1.0)
    j = np.arange(n_in)
    x = (j[None, :] - sample_f[:, None]) / kscale
    w = np.maximum(0.0, 1.0 - np.abs(x))
    w = w / w.sum(1, keepdims=True)
    return w.astype(np.float32)


def _f32(x):
    return np.asarray(x, np.float32)


# ============================= host weight prep =============================

def _prep_block_weights(blocks, L, pfx, wmap):
    for l in range(L):
        g = lambda n: _f32(blocks[n][l])
        ln1w, ln1b = g('ln1_w'), g('ln1_b')
        qkv_w, qkv_b = g('qkv_w'), g('qkv_b')
        proj_w, proj_b = g('proj_w'), g('proj_b')
        ls1, ls2 = g('ls1'), g('ls2')
        ln2w, ln2b = g('ln2_w'), g('ln2_b')
        fc1_w, fc1_b = g('fc1_w'), g('fc1_b')
        fc2_w, fc2_b = g('fc2_w'), g('fc2_b')

        w = ln1w[:, None] * qkv_w
        b = ln1b @ qkv_w + qkv_b
        wq, wk, wv = w[:, :D], w[:, D:2 * D], w[:, 2 * D:]
        bq, bk, bv = b[:D], b[D:2 * D], b[2 * D:]
        sc = HD ** -0.5
        wq, bq = wq * sc, bq * sc
        pw = proj_w * ls1[None, :]
        pb = (proj_b + bv @ proj_w) * ls1
        w1 = ln2w[:, None] * fc1_w
        b1 = ln2b @ fc1_w + fc1_b
        w2 = fc2_w * ls2[None, :]
        b2 = fc2_b * ls2

        def putw(name, arr):
            a = _f32(arr)
            kd = a.shape[0] // 128
            wmap[f"{pfx}{name}{l}"] = np.ascontiguousarray(
                a.reshape(kd, 128, a.shape[1]).transpose(1, 0, 2).astype(bfloat16))

        def putb(name, arr):
            a = _f32(arr)
            md = a.shape[0] // 128
            wmap[f"{pfx}{name}{l}"] = np.ascontiguousarray(a.reshape(md, 128).T)

        putw('wq', wq); putw('wk', wk); putw('wv', wv); putw('wp', pw)
        # fc1 quarters (4, 6, 128, 768); fc2 out-chunks (6, 24, 128, 128)
        wmap[f"{pfx}w1{l}"] = np.ascontiguousarray(
            w1.reshape(6, 128, 4, D).transpose(2, 1, 0, 3).astype(bfloat16))
        wmap[f"{pfx}w2{l}"] = np.ascontiguousarray(
            w2.reshape(24, 128, 6, 128).transpose(2, 1, 0, 3).astype(bfloat16))
        # packed biases [128, 48]: bq bk bp b2 b1
        bias = np.zeros((128, 48), np.float32)
        bias[:, 0:6] = bq.reshape(6, 128).T
        bias[:, 6:12] = bk.reshape(6, 128).T
        bias[:, 12:18] = pb.reshape(6, 128).T
        bias[:, 18:24] = b2.reshape(6, 128).T
        bias[:, 24:48] = b1.reshape(24, 128).T
        wmap[f"{pfx}bias{l}"] = np.ascontiguousarray(bias)


def prep_inputs(inputs):
    sel, ext = inputs['sel'], inputs['ext']
    images = _f32(inputs['images'])
    wmap = {}
    _prep_block_weights(sel['blocks'], 3, 's', wmap)
    _prep_block_weights(ext['blocks'], 12, 'e', wmap)

    wps = _f32(sel['patch_w']).reshape(D, 3 * P * P).T
    wmap['wps'] = np.ascontiguousarray(
        wps.reshape(6, 98, D).transpose(1, 0, 2).astype(bfloat16))
    wpe = _f32(ext['patch_w']).reshape(D, 3 * P * P).T
    wmap['wpe'] = np.ascontiguousarray(
        wpe.reshape(6, 98, D).transpose(1, 0, 2).astype(bfloat16))

    x0add = np.zeros((D, NT_SEL), np.float32)
    x0add[:, 0] = _f32(sel['cls'])[0, 0]
    x0add[:, 1:5] = _f32(sel['reg'])[0].T
    x0add[:, 5:] = (_f32(sel['pos'])[0] + _f32(sel['patch_b'])[None, :]).T
    wmap['x0add'] = np.ascontiguousarray(x0add.reshape(KD, 128, NT_SEL))

    wmap['snormw'] = np.ascontiguousarray(_f32(sel['norm_w']).reshape(KD, 128).T)
    wmap['snormb'] = np.ascontiguousarray(_f32(sel['norm_b']).reshape(KD, 128).T)

    wmap['hw1'] = np.ascontiguousarray(
        _f32(sel['head_w1']).reshape(KD, 128, 4, D).transpose(2, 1, 0, 3).astype(bfloat16))
    wmap['hb1'] = np.ascontiguousarray(_f32(sel['head_b1']).reshape(24, 128).T)
    wmap['hw2'] = np.ascontiguousarray(
        _f32(sel['head_w2']).reshape(24, 128, 16).transpose(1, 0, 2).astype(bfloat16))
    wmap['hb2'] = _f32(sel['head_b2']).reshape(16, 1)

    pos_hr = _f32(ext['pos'])[0] + _f32(ext['patch_b'])[None, :]
    pos_pad = np.zeros((11 * 128, D), np.float32)
    pos_pad[:N_HR] = pos_hr
    wmap['pos_hr'] = np.ascontiguousarray(pos_pad.reshape(11, 128, D))

    nw, nb = _f32(ext['norm_w']), _f32(ext['norm_b'])
    hw = nw[:, None] * _f32(inputs['head_w'])
    hb = nb @ _f32(inputs['head_w']) + _f32(inputs['head_b'])
    wmap['headw'] = np.ascontiguousarray(
        hw.reshape(KD, 128, NCLS).transpose(1, 0, 2).astype(bfloat16))
    wmap['headb'] = hb.reshape(1, NCLS)

    wmap['A518T'] = np.ascontiguousarray(_resize_matrix(HR, LR).T.astype(bfloat16))
    A44 = _resize_matrix(44, G_HR)                      # (37, 44)
    A44T = A44.T                                        # (44, 37)
    wmap['A44i'] = np.ascontiguousarray(
        np.stack([A44T[i::R] for i in range(R)]))       # (4, 11, 37)
    perm = np.array([R * gx + j for j in range(R) for gx in range(G_LR)])
    wmap['A44jg'] = np.ascontiguousarray(A44T[perm])    # (44, 37)
    wmap['iota512'] = np.arange(K_SEL, dtype=np.float32).reshape(1, K_SEL)

    in_maps = []
    for c in range(N_CORES):
        img = images[c % BATCH]
        m = dict(wmap)
        m['img'] = np.ascontiguousarray(img.astype(bfloat16))
        pch = img.reshape(3, G_HR, P, G_HR, P).transpose(0, 2, 4, 1, 3).reshape(588, N_HR)
        pad = np.zeros((588, 11 * 128), np.float32)
        pad[:, :N_HR] = pch
        m['im2hr'] = np.ascontiguousarray(
            pad.reshape(6, 98, 11, 128).transpose(2, 1, 0, 3).astype(bfloat16))
        in_maps.append(m)
    return in_maps


# ============================= device builder ===============================

class KB:
    def __init__(self, nc, tc):
        self.nc, self.tc = nc, tc
        self.inp = {}

    def din(self, name, shape, dtype=F32):
        t = self.nc.dram_tensor(name, shape, dtype, kind="ExternalInput")
        self.inp[name] = t
        return t

    def load_w(self, name, kd, n, col=None, kp=128, sub=None):
        """(kd,kp,n)-DRAM weight -> SBUF [kp,kd,w]. sub indexes a leading dim
        of a 4-D DRAM tensor (contiguous block, single DMA)."""
        nc = self.nc
        s, w = col if col else (0, n)
        t = self.wpool.tile([128, kd, w], BF16, tag="w")
        if sub is not None:
            nc.sync.dma_start(t[:kp], self.inp[name][sub])
        elif col is None:
            nc.sync.dma_start(t[:kp], self.inp[name][:])
        else:
            nc.sync.dma_start(t[:kp], self.inp[name][:, :, s:s + w])
        return t

    def load_b(self, name):
        nc = self.nc
        shp = self.inp[name].shape
        t = self.wpool.tile([128, shp[1]], F32, tag="b")
        nc.sync.dma_start(t[:], self.inp[name][:])
        return t

    # ---------------- layernorm ----------------
    def ln_stats(self, xb, xsq_fn, nt):
        """Returns (r_b, mr_b) psum [128, nt] broadcast tiles.
        xb: bf16 [128, KD, nt]; xsq_fn(k) -> bf16 [128, nt] squares tile."""
        nc, p1, pp = self.nc, self.p1, self.pp
        m_ps = pp.tile([128, nt], F32, tag="pp", name="m_ps")
        q_ps = pp.tile([128, nt], F32, tag="pp", name="q_ps")
        for (s, w) in _nsplit(nt):
            for k in range(KD):
                nc.tensor.matmul(m_ps[:1, s:s + w], self.ones_col_768[:],
                                 xb[:, k, s:s + w], start=(k == 0), stop=(k == KD - 1))
        for k in range(KD):
            xsq = xsq_fn(k)
            for (s, w) in _nsplit(nt):
                nc.tensor.matmul(q_ps[:1, s:s + w], self.ones_col_768[:],
                                 xsq[:, s:s + w], start=(k == 0), stop=(k == KD - 1))
        m2 = p1.tile([1, nt], F32, tag="lnA")
        nc.scalar.square(m2[:], m_ps[:1])
        var = p1.tile([1, nt], F32, tag="lnB")
        nc.vector.scalar_tensor_tensor(var[:], q_ps[:1], 1.0, m2[:], OP.mult, OP.subtract)
        sd = p1.tile([1, nt], F32, tag="lnA")
        nc.scalar.activation(sd[:], var[:], AF.Sqrt, bias=self.eps_row[:], scale=1.0)
        r = p1.tile([1, nt], F32, tag="lnB")
        nc.vector.reciprocal(r[:], sd[:])
        mr = p1.tile([1, nt], F32, tag="lnA")
        nc.vector.tensor_tensor(mr[:], m_ps[:1], r[:], OP.mult)
        r_b = pp.tile([128, nt], F32, tag="pp", name="r_b")
        mr_b = pp.tile([128, nt], F32, tag="pp", name="mr_b")
        for src_row, dst in ((r, r_b), (mr, mr_b)):
            rr = p1.tile([1, nt], F32R, tag="lnr1")
            nc.vector.tensor_copy(rr[:], src_row[:])
            rrb = p1.tile([1, nt], BF16, tag="lnr2")
            nc.vector.tensor_copy(rrb[:], src_row[:])
            for (s, w) in _nsplit(nt):
                if w == 512:
                    nc.tensor.matmul(dst[:, s:s + w], self.ones_row_r[:],
                                     rr[:, s:s + w], start=True, stop=True)
                else:
                    nc.tensor.matmul(dst[:, s:s + w], self.ones_row_b[:],
                                     rrb[:, s:s + w], start=True, stop=True)
        return r_b, mr_b

    def layernorm(self, x, nt, wb=None):
        nc, p1, p2 = self.nc, self.p1, self.p2
        xb = p1.tile([128, KD, nt], BF16, tag="xb")
        for k in range(KD):
            nc.scalar.copy(xb[:, k], x[:, k])

        def xsq_fn(k):
            t = p2.tile([128, nt], BF16, tag="xsq")
            nc.scalar.square(t[:], x[:, k])
            return t

        r_b, mr_b = self.ln_stats(xb, xsq_fn, nt)
        rsb = p1.tile([128, nt], BF16, tag="rsb")
        nc.scalar.copy(rsb[:], r_b[:])
        msb = p1.tile([128, nt], F32, tag="msb")
        nc.scalar.copy(msb[:], mr_b[:])
        xh = p1.tile([128, KD, nt], BF16, tag="xh")
        for k in range(KD):
            tmp = p1.tile([128, nt], BF16, tag="lntmp")
            nc.vector.tensor_tensor(tmp[:], xb[:, k], rsb[:], OP.mult)
            if wb is None:
                nc.vector.tensor_tensor(xh[:, k], tmp[:], msb[:], OP.subtract)
            else:
                w_, b_ = wb
                t2 = p1.tile([128, nt], F32, tag="lntmp2")
                nc.vector.tensor_tensor(t2[:], tmp[:], msb[:], OP.subtract)
                nc.vector.tensor_scalar(xh[:, k], t2[:], w_[:, k:k + 1],
                                        b_[:, k:k + 1], OP.mult, OP.add)
        return xh

    # ---------------- matmul helper ----------------
    def mm_feat(self, w_sb, rhs, nt, m_tot, bias=None, bias_off=0, act=None,
                out=None, out_off=0, out_dt=BF16, out_tag="yT", kd=KD,
                out_pool=None):
        """out[:, out_off+mi, :] = act(W.T @ rhs + bias) feature-major."""
        nc = self.nc
        if out is None:
            out = (out_pool or self.p2).tile([128, m_tot // 128, nt], out_dt, tag=out_tag)
        for mi, (ms, mw) in enumerate(_msplit(m_tot)):
            ps = self.pp.tile([128, nt], F32, tag="pp", name="mmo")
            for (s, w) in _nsplit(nt):
                for k in range(kd):
                    nc.tensor.matmul(ps[:mw, s:s + w], w_sb[:, k, ms:ms + mw],
                                     rhs[:, k, s:s + w],
                                     start=(k == 0), stop=(k == kd - 1))
            if act is not None:
                bb = (bias[:mw, bias_off + mi:bias_off + mi + 1]
                      if bias is not None else 0.0)
                nc.scalar.activation(out[:mw, out_off + mi], ps[:mw], act,
                                     bias=bb, scale=1.0)
            elif bias is not None:
                nc.vector.tensor_scalar(
                    out[:mw, out_off + mi], ps[:mw],
                    bias[:mw, bias_off + mi:bias_off + mi + 1], None, OP.add)
            else:
                nc.vector.tensor_copy(out[:mw, out_off + mi], ps[:mw])
        return out

    # ---------------- transformer block ----------------
    def attention(self, x, xh, nt, pfx, l):
        nc, p1, p2, pp, ps_ = self.nc, self.p1, self.p2, self.pp, self.pss
        mts = _msplit(nt)
        bias = self.load_b(f"{pfx}bias{l}")
        self.cur_bias = bias
        wq = self.load_w(f"{pfx}wq{l}", KD, D)
        qT = self.mm_feat(wq, xh, nt, D, bias=bias[:, 0:6], out_tag="qT",
                          out_pool=self.p1)
        wk = self.load_w(f"{pfx}wk{l}", KD, D)
        kT = self.mm_feat(wk, xh, nt, D, bias=bias[:, 6:12], out_tag="kT",
                          out_pool=self.p1)
        wv = self.load_w(f"{pfx}wv{l}", KD, D)
        v = p1.tile([128, len(mts), NH * (HD + 1)], BF16, tag="v")
        nc.vector.memset(v[:, :, HD::HD + 1], 1.0)
        for mi, (ms, mw) in enumerate(mts):
            vh = v[:, mi].rearrange("p (h e) -> p h e", e=HD + 1)
            for (s, w) in _nsplit(D):
                ps = pp.tile([128, 512], F32, tag="pp", name="vps")
                for k in range(KD):
                    nc.tensor.matmul(ps[:mw, :w], xh[:, k, ms:ms + mw],
                                     wv[:, k, s:s + w],
                                     start=(k == 0), stop=(k == KD - 1))
                nc.vector.tensor_copy(
                    vh[:mw, s // HD:(s + w) // HD, :HD],
                    ps[:mw, :w].rearrange("p (h d) -> p h d", d=HD))

        oT = p1.tile([128, KD, nt], BF16, tag="oT")
        for h in range(NH):
            hs = h * HD
            hc, ho = hs // 128, hs % 128
            expT = p2.tile([128, len(mts), nt], BF16, tag="expT")
            for mi, (ms, mw) in enumerate(mts):
                ps = pp.tile([128, nt], F32, tag="pp", name="scps")
                for (s, w) in _nsplit(nt):
                    nc.tensor.matmul(ps[:mw, s:s + w],
                                     kT[ho:ho + HD, hc, ms:ms + mw],
                                     qT[ho:ho + HD, hc, s:s + w],
                                     start=True, stop=True)
                nc.scalar.activation(expT[:mw, mi], ps[:mw], AF.Exp, scale=1.0)
            ot_ps = pp.tile([128, nt], F32, tag="pp", name="otps")
            for (s, w) in _nsplit(nt):
                for mi, (ms, mw) in enumerate(mts):
                    nc.tensor.matmul(ot_ps[:HD + 1, s:s + w],
                                     v[:mw, mi, h * (HD + 1):(h + 1) * (HD + 1)],
                                     expT[:mw, mi, s:s + w],
                                     start=(mi == 0), stop=(mi == len(mts) - 1))
            recip = p2.tile([1, nt], F32, tag="recip")
            nc.vector.reciprocal(recip[:], ot_ps[HD:HD + 1])
            recr = p2.tile([1, nt], F32R, tag="recipr")
            nc.vector.tensor_copy(recr[:], recip[:])
            recb = p2.tile([1, nt], BF16, tag="recipb")
            nc.vector.tensor_copy(recb[:], recip[:])
            rb_ps = pp.tile([128, nt], F32, tag="pp", name="rbps")
            for (s, w) in _nsplit(nt):
                if w == 512:
                    nc.tensor.matmul(rb_ps[:64, s:s + w], self.ones_row_r[:, :64],
                                     recr[:, s:s + w], start=True, stop=True)
                else:
                    nc.tensor.matmul(rb_ps[:64, s:s + w], self.ones_row_b[:, :64],
                                     recb[:, s:s + w], start=True, stop=True)
            rb_sb = p2.tile([64, nt], F32, tag="rbsb")
            nc.scalar.copy(rb_sb[:], rb_ps[:64])
            nc.vector.tensor_tensor(oT[ho:ho + HD, hc], ot_ps[:HD], rb_sb[:], OP.mult)

        wp = self.load_w(f"{pfx}wp{l}", KD, D)
        bp = self.cur_bias[:, 12:18]
        xn = p2.tile([128, KD, nt], F32, tag="x")
        for mi in range(KD):
            ps = pp.tile([128, nt], F32, tag="pp", name="prps")
            for (s, w) in _nsplit(nt):
                for k in range(KD):
                    nc.tensor.matmul(ps[:, s:s + w], wp[:, k, mi * 128:(mi + 1) * 128],
                                     oT[:, k, s:s + w], start=(k == 0), stop=(k == KD - 1))
            nc.vector.scalar_tensor_tensor(xn[:, mi], ps[:], bp[:, mi:mi + 1],
                                           x[:, mi], OP.add, OP.add)
        return xn

    def mlp(self, x, xh, nt, pfx, l):
        nc, p1, p2, pp = self.nc, self.p1, self.p2, self.pp
        b1 = self.cur_bias[:, 24:48]
        g = p1.tile([128, 24, nt], BF16, tag="m1T")
        for q in range(4):
            w1q = self.load_w(f"{pfx}w1{l}", KD, D, sub=q)
            self.mm_feat(w1q, xh, nt, D, bias=b1, bias_off=q * KD, act=AF.Gelu,
                         out=g, out_off=q * KD)
        b2 = self.cur_bias[:, 18:24]
        xn = p2.tile([128, KD, nt], F32, tag="x")
        for mi in range(KD):
            w2mi = self.load_w(f"{pfx}w2{l}", 24, 128, sub=mi)
            ps = pp.tile([128, nt], F32, tag="pp", name="f2ps")
            for (s, w) in _nsplit(nt):
                for k in range(24):
                    nc.tensor.matmul(ps[:, s:s + w], w2mi[:, k, :],
                                     g[:, k, s:s + w], start=(k == 0), stop=(k == 23))
            nc.vector.scalar_tensor_tensor(xn[:, mi], ps[:], b2[:, mi:mi + 1],
                                           x[:, mi], OP.add, OP.add)
        return xn

    def block(self, x, nt, pfx, l):
        xh = self.layernorm(x, nt)
        x = self.attention(x, xh, nt, pfx, l)
        xh2 = self.layernorm(x, nt)
        x = self.mlp(x, xh2, nt, pfx, l)
        return x


def build(n_sel_layers=3, n_ext_layers=12, dbg=False):
    nc = bacc.Bacc("TRN2", target_bir_lowering=False, debug=True)
    tc_cm = TileContext(nc)
    tc = tc_cm.__enter__()
    kb = KB(nc, tc)

    kb.din('img', (3, HR, HR), BF16)
    kb.din('im2hr', (11, 98, 6, 128), BF16)
    for pfx, L in (('s', n_sel_layers), ('e', n_ext_layers)):
        for l in range(L):
            kb.din(f'{pfx}wq{l}', (128, KD, D), BF16)
            kb.din(f'{pfx}wk{l}', (128, KD, D), BF16)
            kb.din(f'{pfx}wv{l}', (128, KD, D), BF16)
            kb.din(f'{pfx}wp{l}', (128, KD, D), BF16)
            kb.din(f'{pfx}w1{l}', (4, 128, KD, D), BF16)
            kb.din(f'{pfx}w2{l}', (KD, 128, 24, 128), BF16)
            kb.din(f'{pfx}bias{l}', (128, 48))
    kb.din('wps', (98, 6, D), BF16)
    kb.din('wpe', (98, 6, D), BF16)
    kb.din('x0add', (KD, 128, NT_SEL))
    kb.din('snormw', (128, KD)); kb.din('snormb', (128, KD))
    kb.din('hw1', (4, 128, KD, D), BF16); kb.din('hb1', (128, 24))
    kb.din('hw2', (128, 24, 16), BF16); kb.din('hb2', (16, 1))
    kb.din('pos_hr', (11, 128, D))
    kb.din('headw', (128, KD, NCLS), BF16)
    kb.din('headb', (1, NCLS))
    kb.din('A518T', (HR, LR), BF16)
    kb.din('A44i', (R, G_LR, G_HR))
    kb.din('A44jg', (44, G_HR))
    kb.din('iota512', (1, K_SEL))

    logits = nc.dram_tensor("logits", (1, NCLS), F32, kind="ExternalOutput")
    dbg_outs = {}
    if dbg:
        for nm, shp, dt_ in [('d_lo', (3, LR, LR), F32), ('d_up', (1, N_HR), F32),
                             ('d_selx', (KD, 128, NT_SEL), BF16),
                             ('d_rank', (1, 11 * 128), F32),
                             ('d_x0', (KD, 128, NT_EXT), F32)]:
            dbg_outs[nm] = nc.dram_tensor(nm, shp, dt_, kind="ExternalOutput")

    lo_scr = nc.dram_tensor("lo_scr", (3, LR, LR), BF16)
    smap_scr = nc.dram_tensor("smap_scr", (16, N_LR), F32)
    up_scr = nc.dram_tensor("up_scr", (G_HR, G_HR), F32)
    rk_scr = nc.dram_tensor("rk_scr", (11 * 128,), F32)
    te_scr = nc.dram_tensor("te_scr", (11, 128, D), BF16)
    cls_scr = nc.dram_tensor("cls_scr", (1, 2), F32)
    lo3_scr = nc.dram_tensor("lo3_scr", (3, P, P, G_LR, G_LR), BF16)

    with tc.tile_pool(name="const", bufs=1) as cpool, \
         tc.tile_pool(name="p1", bufs=1) as p1, \
         tc.tile_pool(name="p2", bufs=2) as p2, \
         tc.tile_pool(name="wts", bufs=2) as wpool, \
         tc.tile_pool(name="pp", bufs=4, space="PSUM") as pp:

        kb.p1, kb.p2, kb.wpool, kb.pp, kb.pss = p1, p2, wpool, pp, pp
        pstage_cm = tc.tile_pool(name="pstage", bufs=1)
        pstage = pstage_cm.__enter__()

        ones_col_768 = cpool.tile([128, 1], BF16)
        nc.vector.memset(ones_col_768[:], 1.0 / D)
        kb.ones_col_768 = ones_col_768
        ones_col_tok = cpool.tile([128, 1], BF16)
        nc.vector.memset(ones_col_tok[:], 1.0)
        kb.ones_col_tok = ones_col_tok
        ones_row_f = cpool.tile([1, 128], F32)
        nc.vector.memset(ones_row_f[:], 1.0)
        ones_row_r = cpool.tile([1, 128], F32R)
        nc.vector.tensor_copy(ones_row_r[:], ones_row_f[:])
        kb.ones_row_r = ones_row_r
        ones_row_b = cpool.tile([1, 128], BF16)
        nc.vector.tensor_copy(ones_row_b[:], ones_row_f[:])
        kb.ones_row_b = ones_row_b
        eps_row = cpool.tile([1, 1], F32)
        nc.vector.memset(eps_row[:], EPS)
        kb.eps_row = eps_row

        # ---------------- Stage A: bilinear resize (bf16) ----------------
        a518 = pstage.tile([128, 5, LR], BF16, tag="a518")
        hr_chunks = _msplit(HR)
        for ki, (ks, kw) in enumerate(hr_chunks):
            nc.sync.dma_start(a518[:kw, ki], kb.inp['A518T'][ks:ks + kw])
        for c in range(3):
            imgc = pstage.tile([128, 5, HR], BF16, tag="imgc")
            for ki, (ks, kw) in enumerate(hr_chunks):
                nc.sync.dma_start(imgc[:kw, ki], kb.inp['img'][c, ks:ks + kw])
            o1 = pstage.tile([128, 5, LR], BF16, tag="rsz1")
            for wi, (ws, ww) in enumerate(hr_chunks):
                ps = pp.tile([128, LR], F32, tag="pp", name="rszp")
                for ki, (ks, kw) in enumerate(hr_chunks):
                    nc.tensor.matmul(ps[:ww, :], imgc[:kw, ki, ws:ws + ww],
                                     a518[:kw, ki], start=(ki == 0), stop=(ki == 4))
                nc.scalar.copy(o1[:ww, wi], ps[:ww])
            for mi, (ms, mw) in enumerate(_msplit(LR)):
                ps = pp.tile([128, LR], F32, tag="pp", name="rszp2")
                for ki, (ks, kw) in enumerate(hr_chunks):
                    nc.tensor.matmul(ps[:mw, :], o1[:kw, ki, ms:ms + mw],
                                     a518[:kw, ki], start=(ki == 0), stop=(ki == 4))
                lo_sb = pstage.tile([128, LR], BF16, tag="losb")
                nc.scalar.copy(lo_sb[:mw], ps[:mw])
                nc.sync.dma_start(lo_scr[c, ms:ms + mw], lo_sb[:mw])
                if dbg:
                    lo_f = pstage.tile([128, LR], F32, tag="lof")
                    nc.scalar.copy(lo_f[:mw], ps[:mw])
                    nc.sync.dma_start(dbg_outs['d_lo'][c, ms:ms + mw], lo_f[:mw])

        # ---------------- Stage B: hi-res patch embed (token-major) ------
        wpe = kb.load_w('wpe', 6, D, kp=98)
        for mi in range(11):
            im2mi = pstage.tile([98, 6, 128], BF16, tag="im2mi")
            nc.sync.dma_start(im2mi[:], kb.inp['im2hr'][mi])
            posc = pstage.tile([128, D], F32, tag="posc")
            nc.sync.dma_start(posc[:], kb.inp['pos_hr'][mi])
            te_sb = pstage.tile([128, D], BF16, tag="teout")
            for (s, w) in _nsplit(D):
                ps = pp.tile([128, NT_EXT], F32, tag="pp", name="teps")
                for k in range(6):
                    nc.tensor.matmul(ps[:, :w], im2mi[:, k],
                                     wpe[:98, k, s:s + w], start=(k == 0), stop=(k == 5))
                nc.vector.scalar_tensor_tensor(te_sb[:, s:s + w], ps[:, :w], 0.0,
                                               posc[:, s:s + w], OP.add, OP.add)
            nc.sync.dma_start(te_scr[mi], te_sb[:])

        # ---------------- Stage C: selector ViT ----------------
        im2lo = pstage.tile([98, 6, N_LR], BF16, tag="im2lo")
        for c in range(3):
            t_in = pstage.tile([P, G_LR, LR], BF16, tag="perm_in")
            nc.sync.dma_start(t_in[:], lo_scr[c].rearrange(
                "(gy ph) w -> ph gy w", ph=P))
            t_out = pstage.tile([P, P, G_LR, G_LR], BF16, tag="perm_out")
            nc.vector.tensor_copy(
                t_out[:],
                t_in[:].rearrange("p gy (gx pw) -> p pw gy gx", pw=P))
            nc.sync.dma_start(lo3_scr[c], t_out[:])
        for c in range(3):
            for half in range(2):
                nc.sync.dma_start(
                    im2lo[:, c * 2 + half],
                    lo3_scr[c, half * 7:(half + 1) * 7].rearrange(
                        "a b c d -> (a b) (c d)"))
        wps = kb.load_w('wps', 6, D, kp=98)
        x = p2.tile([128, KD, NT_SEL], F32, tag="x")
        x0add_sb = pstage.tile([128, KD, NT_SEL], F32, tag="x0a")
        for k in range(KD):
            nc.sync.dma_start(x0add_sb[:, k], kb.inp['x0add'][k])
            nc.vector.tensor_copy(x[:, k, 0:5], x0add_sb[:, k, 0:5])
        for mi in range(KD):
            ps = pp.tile([128, NT_EXT], F32, tag="pp", name="selte")
            for k in range(6):
                nc.tensor.matmul(ps[:, :N_LR], wps[:98, k, mi * 128:(mi + 1) * 128],
                                 im2lo[:, k], start=(k == 0), stop=(k == 5))
            nc.vector.scalar_tensor_tensor(x[:, mi, 5:], ps[:, :N_LR], 0.0,
                                           x0add_sb[:, mi, 5:], OP.add, OP.add)

        for l in range(n_sel_layers):
            x = kb.block(x, NT_SEL, 's', l)

        snw = pstage.tile([128, KD], F32, tag="snw")
        snb = pstage.tile([128, KD], F32, tag="snb")
        nc.sync.dma_start(snw[:], kb.inp['snormw'][:])
        nc.sync.dma_start(snb[:], kb.inp['snormb'][:])
        xsel = kb.layernorm(x, NT_SEL, wb=(snw, snb))
        xself = pstage.tile([128, KD, 5], F32, tag="xselF")
        for k in range(KD):
            nc.scalar.copy(xself[:, k], xsel[:, k, 0:5])
        if dbg:
            for k in range(KD):
                nc.sync.dma_start(dbg_outs['d_selx'][k], xsel[:, k])

        # ---------------- Stage D: smap head, up, ranks, gather ----------
        b1h = wpool.tile([128, 24], F32, tag="b")
        nc.sync.dma_start(b1h[:], kb.inp['hb1'][:])
        pt = pstage.tile([128, KD, N_LR], BF16, tag="pt")
        for k in range(KD):
            nc.vector.tensor_copy(pt[:, k], xsel[:, k, 5:])
        g1 = p1.tile([128, 24, N_LR], BF16, tag="m1T")
        for q in range(4):
            hw1q = kb.load_w('hw1', KD, D, sub=q)
            kb.mm_feat(hw1q, pt, N_LR, D, bias=b1h, bias_off=q * KD, act=AF.Gelu,
                       out=g1, out_off=q * KD)
        hw2 = kb.load_w('hw2', 24, 16)
        sm_ps = pp.tile([16, N_LR], F32, tag="pp", name="smap")
        for k in range(24):
            nc.tensor.matmul(sm_ps[:, :], hw2[:, k], g1[:, k],
                             start=(k == 0), stop=(k == 23))
        hb2c = pstage.tile([16, 1], F32, tag="hb2c")
        nc.sync.dma_start(hb2c[:], kb.inp['hb2'][:])
        smap_sb = pstage.tile([16, N_LR], F32, tag="smapsb")
        nc.vector.tensor_scalar(smap_sb[:], sm_ps[:], hb2c[:], None, OP.add)
        nc.sync.dma_start(smap_scr[:], smap_sb[:])

        sm2 = pstage.tile([G_LR, R, R, G_LR], F32, tag="sm2")
        for i in range(R):
            nc.sync.dma_start(
                sm2[:, i],
                smap_scr[i * R:(i + 1) * R].rearrange(
                    "j (gy gx) -> gy j gx", gy=G_LR))
        a44i = pstage.tile([G_LR, R, G_HR], F32, tag="a44i")
        for i in range(R):
            nc.sync.dma_start(a44i[:, i], kb.inp['A44i'][i])
        u1_ps = pp.tile([44, G_HR], F32, tag="pp", name="u1")
        for i in range(R):
            nc.tensor.matmul(u1_ps[:, :], sm2[:, i].rearrange("g j x -> g (j x)"),
                             a44i[:, i], start=(i == 0), stop=(i == R - 1))
        u1 = pstage.tile([44, G_HR], F32, tag="u1sb")
        nc.vector.tensor_copy(u1[:], u1_ps[:])
        a44jg = pstage.tile([44, G_HR], F32, tag="a44jg")
        nc.sync.dma_start(a44jg[:], kb.inp['A44jg'][:])
        up_ps = pp.tile([G_HR, G_HR], F32, tag="pp", name="upp")
        nc.tensor.matmul(up_ps[:, :], u1[:], a44jg[:], start=True, stop=True)
        up_sb = pstage.tile([G_HR, G_HR], F32, tag="upsb")
        nc.vector.tensor_copy(up_sb[:], up_ps[:])
        nc.sync.dma_start(up_scr[:], up_sb[:])
        up_flat = up_scr.rearrange("a b -> (a b)")
        if dbg:
            nc.sync.dma_start(dbg_outs['d_up'][:],
                              up_flat.rearrange("(o n) -> o n", o=1))

        up_bcast = pstage.tile([128, N_HR], F32, tag="upbc")
        nc.sync.dma_start(up_bcast[:],
                          up_flat.rearrange("(o n) -> o n", o=1).to_broadcast([128, N_HR]))
        jchunks = _msplit(N_HR)
        NJ = len(jchunks)
        upcols = pstage.tile([128, NJ], F32, tag="upcols")
        for ji, (js, jw) in enumerate(jchunks):
            nc.sync.dma_start(upcols[:jw, ji:ji + 1],
                              up_flat[js:js + jw].rearrange("(p o) -> p o", o=1))
        ranks = pstage.tile([128, NJ], F32, tag="ranks")
        for ji, (js, jw) in enumerate(jchunks):
            scr = pstage.tile([128, N_HR], BF16, tag="cmpscr")
            nc.vector.tensor_scalar(scr[:jw], up_bcast[:jw], upcols[:jw, ji:ji + 1],
                                    0.0, OP.is_gt, OP.add,
                                    accum_out=ranks[:jw, ji:ji + 1])
        if dbg:
            for ji, (js, jw) in enumerate(jchunks):
                nc.sync.dma_start(rk_scr[ji * 128:ji * 128 + jw],
                                  ranks[:jw, ji:ji + 1].rearrange("p o -> (p o)"))
            nc.sync.dma_start(dbg_outs['d_rank'][:],
                              rk_scr.rearrange("(o n) -> o n", o=1))

        iota_bc = pstage.tile([128, K_SEL], F32, tag="iotabc")
        nc.sync.dma_start(iota_bc[:], kb.inp['iota512'][:].to_broadcast([128, K_SEL]))
        st = pstage.tile([128, NJ, K_SEL], BF16, tag="st")
        for ji, (js, jw) in enumerate(jchunks):
            nc.vector.tensor_scalar(st[:jw, ji], iota_bc[:jw],
                                    ranks[:jw, ji:ji + 1], None, OP.is_equal)

        x0 = p2.tile([128, KD, NT_EXT], F32, tag="x")
        for k in range(KD):
            nc.vector.tensor_copy(x0[:, k, 0:5], xself[:, k])
        for mi in range(KD):
            tg = pstage.tile([128, 11, 128], BF16, tag="tegth")
            nc.sync.dma_start(tg[:], te_scr[:, :, mi * 128:(mi + 1) * 128]
                              .rearrange("j p n -> p j n"))
            ps = pp.tile([128, NT_EXT], F32, tag="pp", name="gather")
            for ji, (js, jw) in enumerate(jchunks):
                nc.tensor.matmul(ps[:, :K_SEL], tg[:jw, ji],
                                 st[:jw, ji], start=(ji == 0), stop=(ji == NJ - 1))
            nc.scalar.copy(x0[:, mi, 5:], ps[:, :K_SEL])
        if dbg:
            for k in range(KD):
                nc.sync.dma_start(dbg_outs['d_x0'][k], x0[:, k])

        pstage_cm.__exit__(None, None, None)

        # ---------------- Stage E: extractor + head ----------------
        x = x0
        for l in range(n_ext_layers):
            x = kb.block(x, NT_EXT, 'e', l)

        xbc = p1.tile([128, KD, 1], BF16, tag="xbcls")
        for k in range(KD):
            nc.scalar.copy(xbc[:, k], x[:, k, 0:1])

        def xsqc_fn(k):
            t = p2.tile([128, 1], BF16, tag="xsqc")
            nc.scalar.square(t[:], x[:, k, 0:1])
            return t

        m_ps = pp.tile([1, 2], F32, tag="pp", name="clsstat")
        for k in range(KD):
            nc.tensor.matmul(m_ps[:, 0:1], kb.ones_col_768[:], xbc[:, k],
                             start=(k == 0), stop=(k == KD - 1))
        for k in range(KD):
            xsq = xsqc_fn(k)
            nc.tensor.matmul(m_ps[:, 1:2], kb.ones_col_768[:], xsq[:],
                             start=(k == 0), stop=(k == KD - 1))
        mrow = p1.tile([1, 2], F32, tag="clsrow")
        nc.vector.tensor_copy(mrow[:], m_ps[:])
        m2c = p1.tile([1, 1], F32, tag="clsA")
        nc.scalar.square(m2c[:], mrow[:, 0:1])
        varc = p1.tile([1, 1], F32, tag="clsB")
        nc.vector.scalar_tensor_tensor(varc[:], mrow[:, 1:2], 1.0, m2c[:],
                                       OP.mult, OP.subtract)
        sdc = p1.tile([1, 1], F32, tag="clsA2")
        nc.scalar.activation(sdc[:], varc[:], AF.Sqrt, bias=kb.eps_row[:], scale=1.0)
        rc = p1.tile([1, 1], F32, tag="clsB2")
        nc.vector.reciprocal(rc[:], sdc[:])
        mrc = p1.tile([1, 2], F32, tag="clsC")
        nc.vector.tensor_scalar(mrc[:], mrow[:, 0:2], rc[:], None, OP.mult)
        nc.vector.tensor_copy(mrc[:, 1:2], rc[:])
        nc.sync.dma_start(cls_scr[:], mrc[:])
        clsbc = p1.tile([128, 2], F32, tag="clsbc")
        nc.sync.dma_start(clsbc[:], cls_scr[:].to_broadcast([128, 2]))
        clsh = p1.tile([128, KD, 1], BF16, tag="clsh")
        for k in range(KD):
            tmpc = p2.tile([128, 1], F32, tag="tmpc")
            nc.vector.tensor_tensor(tmpc[:], xbc[:, k], clsbc[:, 1:2], OP.mult)
            nc.vector.tensor_tensor(clsh[:, k], tmpc[:], clsbc[:, 0:1], OP.subtract)
        lg_sb = p1.tile([1, NCLS], F32, tag="logits")
        hbias = p1.tile([1, NCLS], F32, tag="hbias")
        nc.sync.dma_start(hbias[:], kb.inp['headb'][:])
        for (s, w) in _nsplit(NCLS):
            hw = kb.load_w('headw', KD, NCLS, col=(s, w))
            ps = pp.tile([1, 512], F32, tag="pp", name="lgps")
            for k in range(KD):
                nc.tensor.matmul(ps[:, :w], clsh[:, k], hw[:, k, :w],
                                 start=(k == 0), stop=(k == KD - 1))
            nc.vector.tensor_tensor(lg_sb[:, s:s + w], ps[:, :w],
                                    hbias[:, s:s + w], OP.add)
        nc.sync.dma_start(logits[:], lg_sb[:])

    tc_cm.__exit__(None, None, None)
    nc.finalize()
    return nc


_BUILT = None


def kernel(**inputs):
    global _BUILT
    in_maps = prep_inputs(inputs)
    if _BUILT is None:
        _BUILT = build()
    try:
        res = run_bass_kernel_spmd(_BUILT, in_maps, core_ids=list(range(N_CORES)))
    except Exception:
        # transient device hiccups have been observed; one retry
        import time as _t
        _t.sleep(5)
        res = run_bass_kernel_spmd(_BUILT, in_maps, core_ids=list(range(N_CORES)))
    out = np.stack([res.results[c]["logits"][0] for c in range(BATCH)])
    return out.astype(np.float32)
